# revision 5
# baseline (speedup 1.0000x reference)
"""EnsembleFC (E=16 MLPs, 512->512->512->1, relu) on 8 TRN2 NeuronCores.

fp8 DoubleRow kernel: matmuls run in fp8(e4m3) DoubleRow mode (0.5 cyc/row,
4x f32r throughput) with a 3-term error-compensated split, cutting PE time
to 0.75x of the f32r roofline for L1/L2:

  W ~= Wa/32 + Wc/32   (Wa = fp8(32W), Wc = fp8(32(W - Wa/32)), host-side)
  v ~= v_hi + v_lo/32  (activations; produced on device at psum scale)

  pre = Wa.v_hi [+] Wa.v_lo [+] Wc.v_hi      (drop the tiny Wc.v_lo term)

Each DoubleRow matmul sums two K=128 products, so the 3 terms per k-tile
cost 1.5 DR instructions (pairings: (Wa,Wb)@(hi,lo) per kt and
(Wc[kt],Wc[kt+1])@(hi[kt],hi[kt+1]) per kt pair).

Scale bookkeeping (powers of 2, exact):
  x_hi = fp8(x) [1], x_lo = fp8(32(x-x_hi)) [32]       (host)
  L1: (W1a[32],W1b=fp8(W1a/32)[1]) @ (x_hi,x_lo) + W1c[32]@x_hi -> psum@32
  h1_hi = fp8(relu(psum)) [32]                          (ACT, max |32 h1| < 240)
  h1_lo = fp8((psum max 0) - h1_hi) [32]                (DVE stt, one op)
  L2: (W2a[32],W2a) @ (h1_hi,h1_lo) + W2c[32]@h1_hi -> psum@1024
  h2f = f32r(relu(psum) * 2^-10) [1]                    (ACT)
  L3: w3(f32r) @ h2f -> psum@1  (exact f32r path; out + b3 on host)

PSUM (8 banks x 512 f32): banks 2m..2m+1 = L1 member m (4 m-tiles x 256),
banks 4+2m..5+2m = L2 member m; the L3 row [1,256] parks in the mt3 tile
(bank 5+2m cols 256:512) between h2f extraction and the next chunk's
l2(m, mt3), so only that group waits on the ACT copy-out.

PE order per chunk (CH=256 batch cols): L1(m0) L1(m1) L3(m0,c-1) L3(m1,c-1)
L2(m0) L2(m1) -- the L3 block gives the ACT(h1)->DVE(lo) chain of L1(m1)
time to finish before L2(m1) needs h1_lo. ACT h1 passes split per bank pair
so the lo-passes start mid-L1; h2f(m0) on ACT, h2f(m1) on DVE, L3-row
copies on ACT, weight/x DMAs on one SP queue ordered by first use.

Raw Bass (no Tile): same rationale as the f32r baseline -- walrus rejects
instructions with many sync waits; explicit per-engine programs keep each
instruction at <= 2 standalone waits.
"""
import numpy as np
import ml_dtypes

E, D, H, B = 16, 512, 512, 8192
N_CORES = 8
MPC = E // N_CORES          # members per core
KT = D // 128               # k-tiles per 512 contraction
MT = H // 128               # m-tiles per 512 output dim
CH = 256                    # batch columns per chunk (DoubleRow out limit)
NCH = B // CH               # chunks
XBUF = 4                    # x chunk buffering
OSTORE = 4                  # chunks per output store DMA

E4 = ml_dtypes.float8_e4m3

_CACHE = {}


def _build():
    import concourse.bass as bass
    from concourse import mybir

    f32 = mybir.dt.float32
    f32r = mybir.dt.float32r
    fp8 = mybir.dt.float8e4
    DR = mybir.MatmulPerfMode.DoubleRow
    Alu = mybir.AluOpType
    Relu = mybir.ActivationFunctionType.Relu

    nc = bass.Bass("TRN2", target_bir_lowering=False, debug=False,
                   num_devices=N_CORES)

    # host-packed inputs (see kernel() for layouts)
    # w1p slots: 0 = a (fp8(32 W1)), 1 = b (fp8(a/32), pairs with x_lo),
    #            2 = c (fp8(32 (W1 - a/32)))
    # w2p slots: 0 = a, 1 = c (h1_lo pairs with a directly)
    xq = nc.dram_tensor("xq", [128, 2 * KT, B], fp8, kind="ExternalInput").ap()
    w1p = nc.dram_tensor("w1p", [MPC, MT, 128, KT, 3, 128], fp8,
                         kind="ExternalInput").ap()
    w2p = nc.dram_tensor("w2p", [MPC, MT, 128, KT, 2, 128], fp8,
                         kind="ExternalInput").ap()
    w3 = nc.dram_tensor("w3", [128, MPC, KT], f32r, kind="ExternalInput").ap()
    one = nc.dram_tensor("one", [128, 1], f32r, kind="ExternalInput").ap()
    w3sc = nc.dram_tensor("w3sc", [128, KT], f32, kind="ExternalInput").ap()
    out = nc.dram_tensor("out", [MPC, B], f32, kind="ExternalOutput").ap()

    w1ps = [nc.alloc_sbuf_tensor(f"w1ps{m}", [128, MT, KT, 3, 128], fp8).ap()
            for m in range(MPC)]
    w2ps = [nc.alloc_sbuf_tensor(f"w2ps{m}", [128, MT, KT, 2, 128], fp8).ap()
            for m in range(MPC)]
    w3s = nc.alloc_sbuf_tensor("w3s", [128, MPC, KT], f32r).ap()
    ones_s = nc.alloc_sbuf_tensor("ones_s", [128, 1], f32r).ap()
    w3scs = nc.alloc_sbuf_tensor("w3scs", [128, KT], f32).ap()
    s2 = nc.alloc_sbuf_tensor("s2", [128, 2, 512], f32r).ap()
    # member-0 L3 k-reduction on DVE: t_r(c) = sum_kt w3[kt] * h2f[c][kt],
    # double-buffered; PE reduces partitions with one ones-matmul in c+2
    t_r = nc.alloc_sbuf_tensor("t_r", [128, 2, CH], f32r).ap()
    xs = nc.alloc_sbuf_tensor("xs", [128, XBUF, 2, KT, CH], fp8).ap()
    # h1 pair: [:, 0] = hi, [:, 1] = lo (both at scale 32)
    hq = [nc.alloc_sbuf_tensor(f"hq{m}", [128, 2, KT, CH], fp8).ap()
          for m in range(MPC)]
    # h2 in f32r, double-buffered (L3 of chunk c runs during chunk c+1)
    h2f = nc.alloc_sbuf_tensor("h2f", [128, 2, MPC, MT, CH], f32r).ap()
    osb = [nc.alloc_sbuf_tensor(f"osb{m}", [1, NCH * CH], f32).ap()
           for m in range(MPC)]

    ps = nc.alloc_psum_tensor("ps", [128, 8, 512], f32).ap()

    # PE warmup scratch (uninitialized; psum overwritten by start=True later)
    scr = nc.alloc_sbuf_tensor("scr", [128, 128 + CH], f32r).ap()
    N_WARM = _CACHE.get("n_warm_override", 16)

    def l1_psum(m, mt):
        return ps[:, 2 * m + mt // 2, (mt % 2) * 256:(mt % 2) * 256 + 256]

    def l2_psum(m, mt):
        return ps[:, 4 + 2 * m + mt // 2, (mt % 2) * 256:(mt % 2) * 256 + 256]

    # --- tick tables (absolute semaphore counts, mirror emission order) ---
    mmT = {}
    _t = 0
    for c in range(NCH):
        for m in range(MPC):
            for mt in range(MT):
                _t += 1
                mmT[("l1", c, m, mt)] = _t
        if c >= 1:
            _t += 1
            mmT[("l3", c - 1, 0)] = _t
        if c >= 2:
            _t += 1
            mmT[("ones", c - 2)] = _t
        for m in range(MPC):
            for mt in range(MT):
                _t += 1
                mmT[("l2", c, m, mt)] = _t
    _t += 1
    mmT[("l3", NCH - 1, 0)] = _t
    _t += 1
    mmT[("ones", NCH - 2)] = _t
    _t += 1
    mmT[("ones", NCH - 1)] = _t

    # ACT: per chunk: h1(m0,A), h1(m0,B), h1(m1,A), h1(m1,B),
    # [c>=1: cp(c-1,m0), cp(c-1,m1)], h2f(m0); tail: cp(NCH-1, m)
    actT = {}
    _a = 0
    for c in range(NCH):
        for half in range(2):
            _a += 1
            actT[("h1", c, 0, half)] = _a
        for q in range(2):
            _a += 1
            actT[("h1q", c, 1, q)] = _a
        if c >= 1:
            _a += 1
            actT[("cp", c - 1, 0)] = _a
        if c >= 2:
            _a += 1
            actT[("cp", c - 2, 1)] = _a
        _a += 1
        actT[("h2f", c, 0)] = _a
    _a += 1
    actT[("cp", NCH - 1, 0)] = _a
    _a += 1
    actT[("cp", NCH - 2, 1)] = _a
    _a += 1
    actT[("cp", NCH - 1, 1)] = _a

    # DVE: per chunk: lo(m0,A), lo(m0,B), lo(m1,A), lo(m1,B), h2f(m1)
    dveT = {}
    _d = 0
    for c in range(NCH):
        if c >= 1:
            for kt in range(KT):
                _d += 1
                dveT[("h2w", c - 1, kt)] = _d
        for half in range(2):
            _d += 1
            dveT[("lo", c, 0, half)] = _d
        for q in range(2):
            _d += 1
            dveT[("loq", c, 1, q)] = _d
    _d += 1
    dveT[("h2fd", NCH - 1)] = _d

    # Pool: per chunk (c>=1): s2(c-1), t_r(c-1)
    poolT = {}
    _p = 0
    for c in range(1, NCH):
        _p += 1
        poolT[("s2", c - 1)] = _p
        _p += 1
        poolT[("tr", c - 1)] = _p

    with (
        nc.Block() as block,
        nc.semaphore("mm_sem") as mm_sem,
        nc.semaphore("act_sem") as act_sem,
        nc.semaphore("dve_sem") as dve_sem,
        nc.semaphore("w3_sem") as w3_sem,
        nc.semaphore("pool_sem") as pool_sem,
        nc.semaphore("d_sem") as d_sem,
    ):
        x_sems = [nc.alloc_semaphore(f"x_sem{s}") for s in range(XBUF)]
        w1m_sems = [nc.alloc_semaphore(f"w1m_sem{t}") for t in range(MT)]
        w1m1_sems = [nc.alloc_semaphore(f"w1m1_sem{t}") for t in range(MT)]
        w2_sems = [nc.alloc_semaphore(f"w2_sem{m}") for m in range(MPC)]

        def dma_x(sync, c):
            sync.dma_start(
                out=xs[:, c % XBUF],
                in_=xq[:, :, c * CH:(c + 1) * CH],
            ).then_inc(x_sems[c % XBUF], 16)

        @block.sync
        def _(sync: bass.BassEngine):
            # single DMA queue: hand-ordered by first use (chunk-0 L1(m0)
            # slices first, then m1, L2 weights, prefetched x chunks)
            dma_x(sync, 0)
            for mt in range(MT):
                sync.dma_start(out=w1ps[0][:, mt], in_=w1p[0, mt]
                               ).then_inc(w1m_sems[mt], 16)
            for mt in range(MT):
                sync.dma_start(out=w1ps[1][:, mt], in_=w1p[1, mt]
                               ).then_inc(w1m1_sems[mt], 16)
            sync.dma_start(out=w2ps[0], in_=w2p[0].rearrange(
                "mt p kt s f -> p mt kt s f")).then_inc(w2_sems[0], 16)
            dma_x(sync, 1)
            sync.dma_start(out=w2ps[1], in_=w2p[1].rearrange(
                "mt p kt s f -> p mt kt s f")).then_inc(w2_sems[1], 16)
            sync.dma_start(out=w3s, in_=w3).then_inc(w3_sem, 16)
            sync.dma_start(out=ones_s, in_=one).then_inc(w3_sem, 16)
            sync.dma_start(out=w3scs, in_=w3sc).then_inc(w3_sem, 16)
            dma_x(sync, 2)
            dma_x(sync, 3)

            out_r = out.rearrange("m (g ch) -> m g ch", ch=OSTORE * CH)
            for c in range(XBUF, NCH):
                # x slot free once L1 of chunk c-XBUF fully consumed it
                sync.wait_ge(mm_sem, mmT[("l1", c - XBUF, MPC - 1, MT - 1)])
                dma_x(sync, c)
                # trailing output stores, one DMA per OSTORE chunks; lag the
                # store 2 groups behind the pipeline head so the dve wait
                # never blocks SP (x DMAs queue behind it)
                if c % OSTORE == 0 and c >= 2 * OSTORE:
                    g = c // OSTORE - 2
                    sync.wait_ge(
                        act_sem, actT[("cp", (g + 1) * OSTORE - 1, MPC - 1)])
                    for m in range(MPC):
                        sync.dma_start(
                            out=out_r[m:m + 1, g],
                            in_=osb[m][:, g * OSTORE * CH:(g + 1) * OSTORE * CH],
                        ).then_inc(d_sem, 16)
            for g in range(NCH // OSTORE - 2, NCH // OSTORE):
                sync.wait_ge(act_sem, actT[("cp", (g + 1) * OSTORE - 1, MPC - 1)])
                for m in range(MPC):
                    sync.dma_start(
                        out=out_r[m:m + 1, g],
                        in_=osb[m][:, g * OSTORE * CH:(g + 1) * OSTORE * CH],
                    ).then_inc(d_sem, 16)
            sync.wait_ge(d_sem, 16 * MPC * (NCH // OSTORE))

        @block.tensor
        def _(tensor: bass.BassEngine):
            for i in range(N_WARM):
                tensor.matmul(ps[:, 0, 0:256], scr[:, :128], scr[:, 128:128 + CH],
                              start=True, stop=True, skip_group_check=True)

            def l1(c, m):
                slot = c % XBUF
                for mt in range(MT):
                    if mt == 0:
                        if c == 0:
                            tensor.wait_ge(x_sems[0], 16)
                        else:
                            # banks freed once DVE's last lo-pass of c-1 done
                            if m == 0:
                                tensor.wait_ge(dve_sem,
                                               dveT[("lo", c - 1, 0, 1)])
                            else:
                                tensor.wait_ge(dve_sem,
                                               dveT[("loq", c - 1, 1, 1)])
                            if m == 0:
                                tensor.wait_ge(x_sems[slot],
                                               16 * (c // XBUF + 1))
                    if c == 0:
                        tensor.wait_ge(
                            (w1m_sems if m == 0 else w1m1_sems)[mt], 16)
                    pt = l1_psum(m, mt)
                    first = True
                    for s, xsl in ((0, 0), (1, 1), (2, 0)):
                        for kp in range(KT // 2):
                            ins = tensor.matmul(
                                pt,
                                w1ps[m][:, mt, 2 * kp:2 * kp + 2, s, :],
                                xs[:, slot, xsl, 2 * kp:2 * kp + 2, :],
                                start=first, stop=(s == 2 and kp == 1),
                                perf_mode=DR,
                            )
                            first = False
                    ins.then_inc(mm_sem, 1)

            def l3(c, m):
                # m0: f32r psum[1,256] = sum_kt w3^T @ h2f[kt]; row parks in
                # the mt3 tile of L2-m0's bank pair
                if c == 0:
                    tensor.wait_ge(w3_sem, 16)
                tensor.wait_ge(act_sem, actT[("h2f", c, 0)])
                for kt in range(KT):
                    ins = tensor.matmul(
                        ps[0:1, 5 + 2 * m, 256:512],
                        w3s[:, m, kt:kt + 1],
                        h2f[:, c % 2, m, kt, :],
                        start=(kt == 0), stop=(kt == KT - 1),
                    )
                ins.then_inc(mm_sem, 1)

            def ones(c):
                # m1: partition-reduce the Pool-summed t_r(c); row parks in
                # bank 7 mt3, read-clear of h2w(c+1) enforced via dve wait.
                # Tail rows (c >= NCH-2) go to the then-idle bank 1 instead,
                # so they need no h2w WAR wait.
                if c == 0:
                    tensor.wait_ge(w3_sem, 32)
                tensor.wait_ge(pool_sem, poolT[("tr", c)])
                if c < NCH - 2:
                    tensor.wait_ge(dve_sem, dveT[("h2w", c + 1, KT - 1)])
                    row = ps[0:1, 7, 256:512]
                else:
                    row = ps[0:1, 1, 256:512]
                tensor.matmul(
                    row,
                    ones_s,
                    t_r[:, c % 2, :],
                    start=True, stop=True,
                ).then_inc(mm_sem, 1)

            def l2(c, m):
                for mt in range(MT):
                    if mt == 0:
                        if c == 0:
                            tensor.wait_ge(w2_sems[m], 16)
                        # hi-only terms first: gate on the h1 hi pass
                        if m == 0:
                            tensor.wait_ge(act_sem, actT[("h1", c, 0, 1)])
                        else:
                            tensor.wait_ge(act_sem, actT[("h1q", c, 1, 1)])
                    if mt == MT - 1 and c >= 1:
                        # parked L3 row in this tile until copied out
                        if m == 0:
                            tensor.wait_ge(act_sem, actT[("cp", c - 1, 0)])
                        elif c >= 2:
                            tensor.wait_ge(act_sem, actT[("cp", c - 2, 1)])
                    pt = l2_psum(m, mt)
                    first = True
                    for s, hsl in ((1, 0), (0, 0), (0, 1)):
                        if mt == 0 and s == 0 and hsl == 1:
                            # lo-consuming pairs last; gate once per member
                            if m == 0:
                                tensor.wait_ge(dve_sem, dveT[("lo", c, 0, 1)])
                            else:
                                tensor.wait_ge(dve_sem, dveT[("loq", c, 1, 1)])
                        for kp in range(KT // 2):
                            ins = tensor.matmul(
                                pt,
                                w2ps[m][:, mt, 2 * kp:2 * kp + 2, s, :],
                                hq[m][:, hsl, 2 * kp:2 * kp + 2, :],
                                start=first, stop=(s == 0 and hsl == 1
                                                   and kp == 1),
                                perf_mode=DR,
                            )
                            first = False
                    ins.then_inc(mm_sem, 1)

            for c in range(NCH):
                for m in range(MPC):
                    l1(c, m)
                if c >= 1:
                    l3(c - 1, 0)
                if c >= 2:
                    ones(c - 2)
                for m in range(MPC):
                    l2(c, m)
            l3(NCH - 1, 0)
            ones(NCH - 2)
            tensor.wait_ge(dve_sem, dveT[("h2fd", NCH - 1)])
            for kt in range(KT):
                ins = tensor.matmul(
                    ps[0:1, 3, 256:512],
                    w3s[:, 1, kt:kt + 1],
                    h2f[:, (NCH - 1) % 2, 1, kt, :],
                    start=(kt == 0), stop=(kt == KT - 1),
                )
            ins.then_inc(mm_sem, 1)

        @block.scalar
        def _(scalar: bass.BassEngine):
            Copy = bass.mybir.ActivationFunctionType.Copy

            def cp(c, m):
                if m == 0:
                    scalar.wait_ge(mm_sem, mmT[("l3", c, 0)])
                    src = ps[0:1, 5, 256:512]
                else:
                    scalar.wait_ge(mm_sem, mmT[("ones", c)])
                    if c < NCH - 2:
                        src = ps[0:1, 7, 256:512]
                    elif c == NCH - 2:
                        src = ps[0:1, 1, 256:512]
                    else:
                        src = ps[0:1, 3, 256:512]
                scalar.activation(
                    osb[m][0:1, c * CH:(c + 1) * CH],
                    src,
                    Copy,
                ).then_inc(act_sem, 1)

            for c in range(NCH):
                for half in range(2):
                    scalar.wait_ge(mm_sem, mmT[("l1", c, 0, 2 * half + 1)])
                    scalar.activation(
                        hq[0][:, 0, 2 * half:2 * half + 2, :],
                        ps[:, half, :],
                        Relu,
                    ).then_inc(act_sem, 1)
                for q in range(2):
                    scalar.wait_ge(mm_sem, mmT[("l1", c, 1, 2 * q + 1)])
                    scalar.activation(
                        hq[1][:, 0, 2 * q:2 * q + 2, :],
                        ps[:, 2 + q, :],
                        Relu,
                    ).then_inc(act_sem, 1)
                if c >= 1:
                    cp(c - 1, 0)
                if c >= 2:
                    cp(c - 2, 1)
                scalar.wait_ge(mm_sem, mmT[("l2", c, 0, MT - 1)])
                scalar.activation(
                    h2f[:, c % 2, 0],
                    ps[:, 4:6, :],
                    Relu, scale=1.0 / 1024.0,
                ).then_inc(act_sem, 1)
            cp(NCH - 1, 0)
            cp(NCH - 2, 1)
            cp(NCH - 1, 1)

        @block.vector
        def _(vector: bass.BassEngine):
            Alu_ = Alu

            def h2w(c):
                # h2w[kt] = relu(psum) * (w3[kt] * 2^-10), per-partition AP
                vector.wait_ge(mm_sem, mmT[("l2", c, 1, MT - 1)])
                if c == 0:
                    vector.wait_ge(w3_sem, 48)
                for kt in range(KT):
                    vector.tensor_scalar(
                        h2f[:, c % 2, 1, kt, :],
                        ps[:, 6 + kt // 2,
                           (kt % 2) * 256:(kt % 2) * 256 + 256],
                        0.0, w3scs[:, kt:kt + 1],
                        op0=Alu_.max, op1=Alu_.mult,
                    ).then_inc(dve_sem, 1)

            for c in range(NCH):
                if c >= 1:
                    h2w(c - 1)
                for half in range(2):
                    vector.wait_ge(act_sem, actT[("h1", c, 0, half)])
                    vector.scalar_tensor_tensor(
                        hq[0][:, 1, 2 * half:2 * half + 2, :],
                        ps[:, half, :],
                        0.0,
                        hq[0][:, 0, 2 * half:2 * half + 2, :],
                        op0=Alu_.max, op1=Alu_.subtract,
                    ).then_inc(dve_sem, 1)
                for q in range(2):
                    vector.wait_ge(act_sem, actT[("h1q", c, 1, q)])
                    vector.scalar_tensor_tensor(
                        hq[1][:, 1, 2 * q:2 * q + 2, :],
                        ps[:, 2 + q, :],
                        0.0,
                        hq[1][:, 0, 2 * q:2 * q + 2, :],
                        op0=Alu_.max, op1=Alu_.subtract,
                    ).then_inc(dve_sem, 1)
            vector.wait_ge(mm_sem, mmT[("l2", NCH - 1, 1, MT - 1)])
            vector.tensor_scalar(
                h2f[:, (NCH - 1) % 2, 1],
                ps[:, 6:8, :],
                0.0, 1.0 / 1024.0,
                op0=Alu_.max, op1=Alu_.mult,
            ).then_inc(dve_sem, 1)

        @block.gpsimd
        def _(pool: bass.BassEngine):
            # kt-sum of the w3-scaled h2w tiles -> t_r (member 1 L3)
            for c in range(NCH - 1):
                pool.wait_ge(dve_sem, dveT[("h2w", c, KT - 1)])
                pool.tensor_add(
                    s2[:, c % 2, :],
                    h2f[:, c % 2, 1, 0:2, :],
                    h2f[:, c % 2, 1, 2:4, :],
                ).then_inc(pool_sem, 1)
                pool.wait_ge(pool_sem, poolT[("s2", c)])
                pool.tensor_add(
                    t_r[:, c % 2, :],
                    s2[:, c % 2, 0:256],
                    s2[:, c % 2, 256:512],
                ).then_inc(pool_sem, 1)

    return nc


def get_nc():
    if "nc" not in _CACHE:
        _CACHE["nc"] = _build()
    return _CACHE["nc"]


def _q(v):
    return v.astype(E4).astype(np.float32)


def _pack_w(W):
    """W [512,512] f32 -> (wab [MT,128,KT,2,128], wc [MT,128,KT,128]) at
    scale 32, plus the b slot (a/32 for L1, a duplicate for L2) chosen by
    caller via bslot."""
    Wa = _q(32.0 * W)
    Wc = (32.0 * (W - Wa / 32.0)).astype(E4)
    Wa8 = Wa.astype(E4)

    def tile(v):  # [512(in),512(out)] -> [MT,128,KT,128]
        return np.ascontiguousarray(
            v.reshape(KT, 128, MT, 128).transpose(2, 1, 0, 3))

    return tile(Wa8), tile(Wc)


def kernel(x, W1, b1, W2, b2, W3, b3):
    from concourse.bass_utils import run_bass_kernel_spmd

    x = np.asarray(x, dtype=np.float32)
    W1 = np.asarray(W1, dtype=np.float32)
    W2 = np.asarray(W2, dtype=np.float32)
    W3 = np.asarray(W3, dtype=np.float32)
    b1 = np.asarray(b1, dtype=np.float32)
    b2 = np.asarray(b2, dtype=np.float32)
    b3 = np.asarray(b3, dtype=np.float32)

    if np.any(b1 != 0) or np.any(b2 != 0):
        # the fp8 pipeline folds relu(psum) directly; nonzero hidden biases
        # never occur with this model's init -- exact host fallback
        h1 = np.maximum(np.einsum("bi,eih->ebh", x, W1) + b1[:, None, :], 0)
        h2 = np.maximum(np.einsum("ebh,ehk->ebk", h1, W2) + b2[:, None, :], 0)
        o = np.einsum("ebh,eho->ebo", h2, W3) + b3[:, None, :]
        return o.astype(np.float32)

    nc = get_nc()

    # x -> [p, kt, b] hi/lo pair at scales 1 / 32
    xT = np.ascontiguousarray(x.T)                        # [D, B]
    x_hi = xT.astype(E4)
    x_lo = (32.0 * (xT - x_hi.astype(np.float32))).astype(E4)
    xq = np.stack([x_hi.reshape(KT, 128, B).transpose(1, 0, 2),
                   x_lo.reshape(KT, 128, B).transpose(1, 0, 2)], axis=1)
    xq = np.ascontiguousarray(xq).reshape(128, 2 * KT, B)  # [128,2*KT,B]

    in_maps = []
    for core in range(N_CORES):
        w1pk = np.empty((MPC, MT, 128, KT, 3, 128), dtype=E4)
        w2pk = np.empty((MPC, MT, 128, KT, 2, 128), dtype=E4)
        w3m = np.empty((128, MPC, KT), dtype=np.float32)
        for m in range(MPC):
            e = MPC * core + m
            a1, c1 = _pack_w(W1[e])
            w1pk[m, :, :, :, 0, :] = a1
            # b slot: fp8(Wa/32) pairs with x_lo (stored at scale 32)
            w1pk[m, :, :, :, 1, :] = (a1.astype(np.float32) / 32.0).astype(E4)
            w1pk[m, :, :, :, 2, :] = c1
            a2, c2 = _pack_w(W2[e])
            # h1_lo is already at scale 32, so it pairs with the a slot
            w2pk[m, :, :, :, 0, :] = a2
            w2pk[m, :, :, :, 1, :] = c2
            w3m[:, m, :] = W3[e, :, 0].reshape(KT, 128).T
        w3sc = np.ascontiguousarray(
            W3[MPC * core + 1, :, 0].reshape(KT, 128).T / 1024.0
        ).astype(np.float32)
        in_maps.append({"xq": xq, "w1p": w1pk, "w2p": w2pk, "w3": w3m,
                        "one": np.ones((128, 1), dtype=np.float32),
                        "w3sc": w3sc})

    res = run_bass_kernel_spmd(nc, in_maps, list(range(N_CORES)))
    out = np.concatenate([r["out"] for r in res.results], axis=0)  # [E, B]
    out = out + b3.reshape(E, 1)
    return out.reshape(E, B, 1).astype(np.float32)
